# revision 14
# baseline (speedup 1.0000x reference)
"""DGCNN forward on 8 TRN2 NeuronCores — data-parallel over batch.

kernel(**inputs) takes the full inputs from setup_inputs() and returns the
full [8, 40] output. One sample per core; the whole network runs on-chip.

v2 design (everything transposed, features-on-partitions):
  EdgeConv(x)[n,o] = lrelu( max_k proj[idx[n,k],o] + a[n,o] ) computed as
  mT[o,n] via:
   - pd = [2xT;1;d2]^T . [xT;-d2;-1] per 128-row block (fp32r matmuls,
     4x faster than fp32), diagonal forced to +2^100 via an extra
     identity matmul so rank-0 is always self.
   - top-24 via two-level chunked max8 with 12-bit index packing into the
     fp32 mantissa (one max8+max_index per 256-col chunk, then 3 max8
     rounds over 64 packed candidates; indices extracted by bit masking).
   - neighbor gather: gpsimd indirect_copy (SBUF column gather per
     16-partition group) on projT [O, N] with two wrapped index lists
     (ranks 0-15 and 4-19), built via PE transpose + replication matmul.
   - k-max: one strided DVE reduce over both gathered halves; +aT; leaky
     ReLU on Act (Prelu) written straight into the next layer's xT (f32r).
  Head: x5/fc matmuls in fp32r; max+mean pooling fused into PSUM->SBUF
  activation copies (accum_out) and DVE reduces.
"""
import numpy as np
from contextlib import ExitStack

import concourse.bass as bass
import concourse.mybir as mybir
import concourse.tile as tile
from concourse import bacc
from concourse.bass_utils import run_bass_kernel_spmd
from concourse.masks import make_identity

P = 128
N = 2048
NBLK = N // P
K = 20
EPS = 1e-5
NEG = -1e30
BIG = float(2.0 ** 100)
CH = 8            # stage-1 chunks
CW = N // CH      # 256
MASK_KEEP = 0xFFFFF800
MASK_IDX = 0x7FF
f32 = mybir.dt.float32
f32r = mybir.dt.float32r
u16 = mybir.dt.uint16
u32 = mybir.dt.uint32
AF = mybir.ActivationFunctionType
ALU = mybir.AluOpType

# (C_in, O) per EdgeConv layer
LAYERS = [(3, 64), (64, 64), (64, 128), (128, 256)]


# ---------------------------------------------------------------- program ---
def build_program():
    nc = bacc.Bacc("TRN2", target_bir_lowering=False, debug=False)

    xT_d = nc.dram_tensor("xT", [3, N], f32r, kind="ExternalInput")
    pw_d, aw_d = [], []
    for li, (C, O) in enumerate(LAYERS):
        pw_d.append(nc.dram_tensor(f"pw{li}", [C, O], f32r, kind="ExternalInput"))
        aw_d.append((nc.dram_tensor(f"aw{li}", [C, O], f32r, kind="ExternalInput"),
                     nc.dram_tensor(f"awt{li}", [1, O], f32r, kind="ExternalInput")))
    w5_d = [nc.dram_tensor(f"w5c{c}", [64 if c < 2 else 128, 1024], f32r,
                           kind="ExternalInput") for c in range(5)]
    t5_d = nc.dram_tensor("t5", [1, 1024], f32r, kind="ExternalInput")
    wf1_d = nc.dram_tensor("wf1", [2048, 512], f32r, kind="ExternalInput")
    t6_d = nc.dram_tensor("t6", [1, 512], f32, kind="ExternalInput")
    wf2_d = nc.dram_tensor("wf2", [512, 256], f32r, kind="ExternalInput")
    t7_d = nc.dram_tensor("t7", [1, 256], f32, kind="ExternalInput")
    wf3_d = nc.dram_tensor("wf3", [256, 40], f32r, kind="ExternalInput")
    t8_d = nc.dram_tensor("t8", [1, 40], f32, kind="ExternalInput")
    cbase_d = nc.dram_tensor("cbase", [P, CH * 8], u32, kind="ExternalInput")
    onesr_d = nc.dram_tensor("onesr", [1, N], f32r, kind="ExternalInput")
    negr_d = nc.dram_tensor("negr", [1, N], f32r, kind="ExternalInput")
    onescol_d = nc.dram_tensor("onescol", [P, 1], f32r, kind="ExternalInput")
    rep_d = nc.dram_tensor("rep", [16, P], f32, kind="ExternalInput")
    identr_d = nc.dram_tensor("identr", [P, P], f32r, kind="ExternalInput")
    bigid_d = nc.dram_tensor("bigid", [P, P], f32r, kind="ExternalInput")
    h1D = nc.dram_tensor("h1D", [1, 512], f32r)
    h2D = nc.dram_tensor("h2D", [1, 256], f32r)
    out_d = nc.dram_tensor("out", [1, 40], f32, kind="ExternalOutput")

    with tile.TileContext(nc) as tc, ExitStack() as ctx:
        persist = ctx.enter_context(tc.tile_pool(name="persist", bufs=1))

        # persistent feature tiles (transposed, f32r)
        xT0 = persist.tile([3, N], f32r)
        nc.sync.dma_start(xT0[:], xT_d[:])
        x1T = persist.tile([64, N], f32r)
        x2T = persist.tile([64, N], f32r)
        x3T = persist.tile([P, N], f32r)
        x4Ta = persist.tile([P, N], f32r)
        x4Tb = persist.tile([P, N], f32r)
        A_hi = persist.tile([P, N], f32r)   # 2*xT of current layer
        A_lo = persist.tile([2, N], f32r)   # [ones; d2]
        B_lo = persist.tile([2, N], f32r)   # [-d2; -ones]
        sq = persist.tile([P, N], f32r)
        ones_row = persist.tile([1, N], f32r)
        ones_col = persist.tile([P, 1], f32r)
        d2row = persist.tile([1, N], f32r)
        nc.sync.dma_start(ones_row[:], onesr_d[:])
        nc.sync.dma_start(ones_col[:], onescol_d[:])
        nc.sync.dma_start(A_lo[0:1, :], onesr_d[:])
        nc.sync.dma_start(B_lo[1:2, :], negr_d[:])
        ident = persist.tile([P, P], f32)
        make_identity(nc, ident[:])
        identr = persist.tile([P, P], f32r)
        nc.sync.dma_start(identr[:], identr_d[:])
        bigid = persist.tile([P, P], f32r)
        nc.sync.dma_start(bigid[:], bigid_d[:])
        cbase = persist.tile([P, CH * 8], u32)
        nc.sync.dma_start(cbase[:], cbase_d[:])
        rep = persist.tile([16, P], f32)
        nc.sync.dma_start(rep[:], rep_d[:])
        projTa = persist.tile([P, N], f32)   # layer projT (rows 0..127)
        projTb = persist.tile([P, N], f32)   # L4 rows 128..255
        aTa = persist.tile([P, N], f32)
        aTb = persist.tile([P, N], f32)
        nc.gpsimd.memset(projTa[:], 0.0)
        nc.gpsimd.memset(projTb[:], 0.0)
        nc.gpsimd.memset(aTa[:], 0.0)
        nc.gpsimd.memset(aTb[:], 0.0)

        # conv weights in SBUF
        pw_sb, aw_sb = [], []
        for li, (C, O) in enumerate(LAYERS):
            t = persist.tile([C, O], f32r, name=f"pwsb{li}")
            nc.sync.dma_start(t[:], pw_d[li][:])
            pw_sb.append(t)
            t = persist.tile([C, O], f32r, name=f"awsb{li}")
            nc.sync.dma_start(t[:], aw_d[li][0][:])
            tt = persist.tile([1, O], f32r, name=f"awtsb{li}")
            nc.sync.dma_start(tt[:], aw_d[li][1][:])
            aw_sb.append((t, tt))

        # ---------------- layer phase (own pools, closed before head) -------
        les = ExitStack()
        pdp = les.enter_context(tc.tile_pool(name="pd", bufs=2))
        tkp = les.enter_context(tc.tile_pool(name="topk", bufs=2))
        gtp = les.enter_context(tc.tile_pool(name="gt", bufs=2))
        psPD = les.enter_context(tc.tile_pool(name="psPD", bufs=2, space="PSUM"))
        psMM = les.enter_context(tc.tile_pool(name="psMM", bufs=3, space="PSUM"))

        layer_in = [(xT0, None), (x1T, None), (x2T, None), (x3T, None)]
        layer_out = [[(x1T, 64)], [(x2T, 64)], [(x3T, P)], [(x4Ta, P), (x4Tb, P)]]

        for li, (C, O) in enumerate(LAYERS):
            xT = layer_in[li][0][0:C, :]
            # ---- layer prep: squares, d2 row, A_hi, proj/a transposed
            nc.scalar.activation(sq[0:C, :], xT[:, :], AF.Square)
            for c4 in range(4):
                d2ps = psMM.tile([1, 512], f32, tag="mm", name=f"d2ps{li}_{c4}")
                nc.tensor.matmul(d2ps[:], ones_col[0:C, :], sq[0:C, bass.ts(c4, 512)],
                                 start=True, stop=True)
                nc.vector.tensor_copy(d2row[:, bass.ts(c4, 512)], d2ps[:])
                nc.vector.tensor_scalar(out=B_lo[0:1, bass.ts(c4, 512)], in0=d2ps[:],
                                        scalar1=-1.0, scalar2=None, op0=ALU.mult)
            nc.sync.dma_start(A_lo[1:2, :], d2row[:])
            nc.vector.tensor_scalar(out=A_hi[0:C, :], in0=xT[:, :], scalar1=2.0,
                                    scalar2=None, op0=ALU.mult)

            # projT/aT for the whole layer
            ngrp = (O + P - 1) // P
            for g in range(ngrp):
                rows = min(P, O - g * P)
                pdst = projTa if g == 0 else projTb
                adst = aTa if g == 0 else aTb
                for c4 in range(4):
                    cs = bass.ts(c4, 512)
                    pp = psMM.tile([P, 512], f32, tag="mm", name=f"pj{li}_{g}_{c4}")
                    nc.tensor.matmul(pp[0:rows, :], pw_sb[li][:, bass.ds(g * P, rows)],
                                     xT[:, cs], start=True, stop=True)
                    nc.scalar.activation(pdst[0:rows, cs], pp[0:rows, :], AF.Copy)
                    pa = psMM.tile([P, 512], f32, tag="mm", name=f"pa{li}_{g}_{c4}")
                    nc.tensor.matmul(pa[0:rows, :], aw_sb[li][0][:, bass.ds(g * P, rows)],
                                     xT[:, cs], start=True, stop=False)
                    nc.tensor.matmul(pa[0:rows, :],
                                     aw_sb[li][1][:, bass.ds(g * P, rows)],
                                     ones_row[:, cs], start=False, stop=True)
                    nc.scalar.activation(adst[0:rows, cs], pa[0:rows, :], AF.Copy)

            # ---- per block
            for b in range(NBLK):
                bs = bass.ts(b, P)
                # pd in PSUM (2 halves of 1024), diag forced to BIG
                pd_sb = pdp.tile([P, N], f32, tag="pd", name=f"pd{li}_{b}")
                for h in range(2):
                    ph = psPD.tile([P, 1024], f32, tag="pdps", name=f"ph{li}_{b}_{h}")
                    for c2 in range(2):
                        cbeg = h * 1024 + c2 * 512
                        ms = bass.ds(cbeg, 512)
                        po = ph[:, bass.ts(c2, 512)]
                        diag_here = cbeg <= b * P < cbeg + 512
                        nc.tensor.matmul(po, A_hi[0:C, bs], xT[:, ms],
                                         start=True, stop=False)
                        nc.tensor.matmul(po, A_lo[:, bs], B_lo[:, ms],
                                         start=False, stop=not diag_here)
                        if diag_here:
                            off = b * P - cbeg
                            nc.tensor.matmul(po[:, bass.ds(off, P)],
                                             bigid[:], identr[:],
                                             start=False, stop=True)
                    nc.scalar.activation(pd_sb[:, bass.ts(h, 1024)], ph[:], AF.Copy)

                # two-level packed topk
                cv = tkp.tile([P, CH * 8], f32, tag="cv", name=f"cv{li}_{b}")
                ci = tkp.tile([P, CH * 8], u32, tag="ci", name=f"ci{li}_{b}")
                for c in range(CH):
                    nc.vector.max(out=cv[:, bass.ts(c, 8)], in_=pd_sb[:, bass.ts(c, CW)])
                    nc.vector.max_index(out=ci[:, bass.ts(c, 8)],
                                        in_max=cv[:, bass.ts(c, 8)],
                                        in_values=pd_sb[:, bass.ts(c, CW)])
                nc.vector.tensor_tensor(out=ci[:], in0=ci[:], in1=cbase[:], op=ALU.add)
                cvu = cv[:].bitcast(u32)
                nc.vector.tensor_scalar(out=cvu, in0=cvu, scalar1=MASK_KEEP,
                                        scalar2=None, op0=ALU.bitwise_and)
                nc.vector.tensor_tensor(out=cvu, in0=cvu, in1=ci[:], op=ALU.bitwise_or)
                top = tkp.tile([P, 24], f32, tag="top", name=f"top{li}_{b}")
                for r in range(3):
                    nc.vector.max(out=top[:, bass.ts(r, 8)], in_=cv[:])
                    if r < 2:
                        nc.vector.match_replace(out=cv[:], in_to_replace=top[:, bass.ts(r, 8)],
                                                in_values=cv[:], imm_value=NEG)
                idxu = tkp.tile([P, 24], u32, tag="idxu", name=f"idxu{li}_{b}")
                nc.vector.tensor_scalar(out=idxu[:], in0=top[:].bitcast(u32),
                                        scalar1=MASK_IDX, scalar2=None,
                                        op0=ALU.bitwise_and)

                # build wrapped u16 index lists (A: ranks 0-15, B: ranks 4-19)
                idxs_sb = []
                for (lname, lo) in (("A", 0), ("B", 4)):
                    idxf = tkp.tile([P, 16], f32, tag="idxf", name=f"ixf{lname}{li}_{b}")
                    nc.vector.tensor_copy(idxf[:], idxu[:, lo:lo + 16])
                    pt = psMM.tile([16, P], f32, tag="mm", name=f"pt{lname}{li}_{b}")
                    nc.tensor.transpose(pt[:], idxf[:], ident[:])
                    itf = tkp.tile([16, P], f32, tag="itf", name=f"itf{lname}{li}_{b}")
                    nc.scalar.activation(itf[:], pt[:], AF.Copy)
                    bps = psMM.tile([P, P], f32, tag="mm", name=f"bps{lname}{li}_{b}")
                    nc.tensor.matmul(bps[:], rep[:], itf[:], start=True, stop=True)
                    isb = tkp.tile([P, P], u16, tag="isb", name=f"isb{lname}{li}_{b}")
                    nc.vector.tensor_copy(isb[:], bps[:])
                    idxs_sb.append(isb)

                # gather + reduce + combine per 128-row group of O
                for g in range(ngrp):
                    rows = min(P, O - g * P)
                    dsrc = projTa if g == 0 else projTb
                    asrc = aTa if g == 0 else aTb
                    gt = gtp.tile([P, 2, N], f32, tag="gt", name=f"gt{li}_{b}_{g}")
                    for l in range(2):
                        for hv in range(2):
                            nc.gpsimd.indirect_copy(
                                gt[:, l, bass.ts(hv, 1024)], dsrc[:],
                                idxs_sb[l][:, bass.ts(hv, 64)],
                                i_know_ap_gather_is_preferred=True)
                    m = tkp.tile([P, P], f32, tag="m", name=f"m{li}_{b}_{g}")
                    nc.vector.tensor_reduce(
                        m[:], gt[:].rearrange("d l (p k) -> d p l k", k=16),
                        axis=mybir.AxisListType.XY, op=ALU.max)
                    nc.vector.tensor_tensor(out=m[:], in0=m[:], in1=asrc[:, bs],
                                            op=ALU.add)
                    dst, _ = layer_out[li][g]
                    nc.scalar.activation(dst[0:rows, bs], m[0:rows, :], AF.Prelu,
                                         alpha=0.2)

        les.close()

        # ---------------- head phase --------------------------------------
        psB = ctx.enter_context(tc.tile_pool(name="psB", bufs=2, space="PSUM"))
        hwork = ctx.enter_context(tc.tile_pool(name="hwork", bufs=2))
        hp = ctx.enter_context(tc.tile_pool(name="hpersist", bufs=1))

        w5_sb = []
        for c in range(5):
            rows = 64 if c < 2 else 128
            t = hp.tile([rows, 1024], f32r, name=f"w5sb{c}")
            nc.sync.dma_start(t[:], w5_d[c][:])
            w5_sb.append(t)
        t5_sb = hp.tile([1, 1024], f32r)
        nc.sync.dma_start(t5_sb[:], t5_d[:])
        wf2_sb = hp.tile([P, 4 * 256], f32r)
        nc.sync.dma_start(wf2_sb[:].rearrange("p (c f) -> p c f", c=4),
                          wf2_d[:].rearrange("(c p) f -> p c f", p=P))
        wf3_sb = hp.tile([P, 2 * 40], f32r)
        nc.sync.dma_start(wf3_sb[:].rearrange("p (c f) -> p c f", c=2),
                          wf3_d[:].rearrange("(c p) f -> p c f", p=P))
        t6_sb = hp.tile([1, 512], f32)
        nc.sync.dma_start(t6_sb[:], t6_d[:])
        t7_sb = hp.tile([1, 256], f32)
        nc.sync.dma_start(t7_sb[:], t7_d[:])
        t8_sb = hp.tile([1, 40], f32)
        nc.sync.dma_start(t8_sb[:], t8_d[:])

        # x5 = lrelu(w5m.T @ xc + t5) computed as 8 row-chunks of [128, N];
        # per chunk: col-max into gmax[:, r], col-sum into gsum[:, r]
        chunks = [x1T, x2T, x3T, x4Ta, x4Tb]
        gmax = hp.tile([P, 8], f32r)
        gsum = hp.tile([P, 8], f32r)
        for r in range(8):
            rsl = bass.ts(r, P)
            cmax = hwork.tile([P, 4], f32, tag="cmax", name=f"cmax{r}")
            csum = hwork.tile([P, 4], f32, tag="csum", name=f"csum{r}")
            for cc in range(4):
                cs = bass.ts(cc, 512)
                px = psB.tile([P, 512], f32, tag="ps", name=f"px{r}_{cc}")
                for c, chn in enumerate(chunks):
                    nc.tensor.matmul(px[:], w5_sb[c][:, rsl], chn[:, cs],
                                     start=(c == 0), stop=False)
                nc.tensor.matmul(px[:], t5_sb[:, rsl], ones_row[:, cs],
                                 start=False, stop=True)
                x5h = hwork.tile([P, 512], f32, tag="x5h", name=f"x5h{r}_{cc}")
                nc.scalar.activation(x5h[:], px[:], AF.Prelu, alpha=0.2,
                                     accum_out=csum[:, cc:cc + 1])
                nc.vector.tensor_reduce(cmax[:, cc:cc + 1], x5h[:],
                                        axis=mybir.AxisListType.X, op=ALU.max)
            nc.vector.tensor_reduce(gmax[:, r:r + 1], cmax[:],
                                    axis=mybir.AxisListType.X, op=ALU.max)
            with nc.allow_low_precision(reason="f32r col-sum for fc1 operand"):
                nc.vector.tensor_reduce(gsum[:, r:r + 1], csum[:],
                                        axis=mybir.AxisListType.X, op=ALU.add)

        # ---- fc1: [1,2048]@[2048,512]; mean half of wf1 pre-scaled by 1/N
        f1ps = psB.tile([1, 512], f32, tag="f1ps", bufs=1)
        for c in range(16):
            wchunk = hwork.tile([P, 512], f32r, tag="wf1c", name=f"wf1c{c}")
            nc.sync.dma_start(wchunk[:], wf1_d[bass.ts(c, P), :])
            src = gmax if c < 8 else gsum
            col = c % 8
            nc.tensor.matmul(f1ps[:], src[:, col:col + 1], wchunk[:],
                             start=(c == 0), stop=(c == 15))
        h1 = hp.tile([1, 512], f32)
        nc.vector.tensor_tensor(out=h1[:], in0=f1ps[:], in1=t6_sb[:], op=ALU.add)
        h1r = hp.tile([1, 512], f32r)
        nc.scalar.activation(h1r[:], h1[:], AF.Prelu, alpha=0.2)
        nc.sync.dma_start(h1D[:], h1r[:])
        h1col = hp.tile([P, 4], f32r)
        nc.sync.dma_start(h1col[:], h1D[:].rearrange("a (c p) -> (a p) c", p=P))

        # ---- fc2: [1,512]@[512,256]
        f2ps = psB.tile([1, 256], f32, tag="f2ps", bufs=1)
        for c in range(4):
            nc.tensor.matmul(f2ps[:], h1col[:, c:c + 1], wf2_sb[:, bass.ts(c, 256)],
                             start=(c == 0), stop=(c == 3))
        h2 = hp.tile([1, 256], f32)
        nc.vector.tensor_tensor(out=h2[:], in0=f2ps[:], in1=t7_sb[:], op=ALU.add)
        h2r = hp.tile([1, 256], f32r)
        nc.scalar.activation(h2r[:], h2[:], AF.Prelu, alpha=0.2)
        nc.sync.dma_start(h2D[:], h2r[:])
        h2col = hp.tile([P, 2], f32r)
        nc.sync.dma_start(h2col[:], h2D[:].rearrange("a (c p) -> (a p) c", p=P))

        # ---- fc3: [1,256]@[256,40]
        f3ps = psB.tile([1, 40], f32, tag="f3ps", bufs=1)
        for c in range(2):
            nc.tensor.matmul(f3ps[:], h2col[:, c:c + 1], wf3_sb[:, bass.ts(c, 40)],
                             start=(c == 0), stop=(c == 1))
        ofin = hp.tile([1, 40], f32)
        nc.vector.tensor_tensor(out=ofin[:], in0=f3ps[:], in1=t8_sb[:], op=ALU.add)
        nc.sync.dma_start(out_d[:], ofin[:])

    nc.compile()
    return nc


# ------------------------------------------------------------- host glue ---
def _fold_params(I):
    def conv(w, bn):
        O, twoC = w.shape
        C = twoC // 2
        g, b, m, v = bn
        s = g / np.sqrt(v + EPS)
        t = b - m * s
        wd, wc = w[:, :C], w[:, C:]
        Pw = (wd * s[:, None]).T.astype(np.float32)
        Aw = ((wc - wd) * s[:, None]).T.astype(np.float32)
        return (np.ascontiguousarray(Pw), np.ascontiguousarray(Aw),
                np.ascontiguousarray(t[None, :].astype(np.float32)))

    out = {}
    for li, wk, bk in [(0, "w1", "bn1"), (1, "w2", "bn2"), (2, "w3", "bn3"),
                       (3, "w4", "bn4")]:
        pw, aw, awt = conv(I[wk], I[bk])
        out[f"pw{li}"] = pw
        out[f"aw{li}"] = aw
        out[f"awt{li}"] = awt

    def fc(w, bn):
        g, b, m, v = bn
        s = g / np.sqrt(v + EPS)
        t = b - m * s
        return (np.ascontiguousarray((w * s[:, None]).T.astype(np.float32)),
                t.astype(np.float32))

    w5m, t5 = fc(I["w5"], I["bn5"])
    bnds = [0, 64, 128, 256, 384, 512]
    for c in range(5):
        out[f"w5c{c}"] = np.ascontiguousarray(w5m[bnds[c]:bnds[c + 1]])
    out["t5"] = t5[None, :]
    wf1, t6 = fc(I["wl1"], I["bn6"])
    wf1 = wf1.copy()
    wf1[1024:, :] *= 1.0 / N          # mean-half pre-scaled by 1/N
    out["wf1"], out["t6"] = wf1, t6[None, :]
    g7, b7, m7, v7 = I["bn7"]
    s7 = g7 / np.sqrt(v7 + EPS)
    t7 = b7 - m7 * s7
    out["wf2"] = np.ascontiguousarray((I["wl2"] * s7[:, None]).T.astype(np.float32))
    out["t7"] = (I["bl2"] * s7 + t7).astype(np.float32)[None, :]
    out["wf3"] = np.ascontiguousarray(I["wl3"].T.astype(np.float32))
    out["t8"] = I["bl3"].astype(np.float32)[None, :]

    # constants
    cb = np.broadcast_to((np.arange(CH, dtype=np.uint32) * CW)[:, None],
                         (CH, 8)).reshape(1, CH * 8)
    out["cbase"] = np.ascontiguousarray(np.broadcast_to(cb, (P, CH * 8)))
    rep = np.zeros((16, P), np.float32)
    rep[np.arange(P) % 16, np.arange(P)] = 1.0
    out["rep"] = rep
    out["identr"] = np.eye(P, dtype=np.float32)
    out["bigid"] = (np.eye(P, dtype=np.float32) * BIG).astype(np.float32)
    out["onesr"] = np.ones((1, N), np.float32)
    out["negr"] = -np.ones((1, N), np.float32)
    out["onescol"] = np.ones((P, 1), np.float32)
    return out


_NC_CACHE = None


def get_nc():
    global _NC_CACHE
    if _NC_CACHE is None:
        _NC_CACHE = build_program()
    return _NC_CACHE


def make_in_maps(inputs):
    I = {k: np.asarray(v) for k, v in inputs.items()}
    params = _fold_params(I)
    B = I["x"].shape[0]
    in_maps = []
    for b in range(B):
        m = dict(params)
        m["xT"] = np.ascontiguousarray(I["x"][b].T.astype(np.float32))
        in_maps.append(m)
    return in_maps


def kernel(**inputs):
    nc = get_nc()
    in_maps = make_in_maps(inputs)
    res = run_bass_kernel_spmd(nc, in_maps, list(range(len(in_maps))))
    return np.stack([r["out"][0] for r in res.results]).astype(np.float32)
